# revision 13
# baseline (speedup 1.0000x reference)
"""Trainium2 Bass kernel for log-softmax multi-head attention (8 NeuronCores).

Reference computation (per batch):
    qkv = x @ w_qkv ; q,k,v per head
    dots = scale * q @ k^T ; attn = log_softmax(dots)
    out = attn @ v  -> merge heads -> out @ w_out + b_out + x

Key algebraic identity: log_softmax is linear in the scores minus a row
constant:  attn = scale*dots - lse  with  lse_i = logsumexp_j(scale*dots_ij).
Therefore
    out_head = scale * q @ (k^T v) - lse (x) colsum(v)
which removes the O(n^2 d) attention-apply; only the lse pass is O(n^2).

Sharding: 8 cores = 2 batches x 4 query-quarters. Every core computes k/v for
its full batch (duplicated across the 4 cores of a batch) and q / lse / output
for its own 1024 query rows -> outputs are disjoint, no collectives.

Precision: the n^2-sized work (q/k productions, scores) runs in bf16 on the
TensorEngine (1 cycle/row vs 2 for f32r / 4 for fp32); the lse-correction
rank-1 term and the final projection stay fp32/f32r. Accumulation is always
fp32 in PSUM.
"""

import numpy as np

B, N, D = 2, 4096, 512
H, DH = 8, 64
SCALE = DH**-0.5
NQ = N // 4  # own query rows per core
NT = N // 128  # 32 key tiles
QT = NQ // 128  # 8 own row tiles

_GRAPH_CACHE = {}


def _build_graph():
    import concourse.bass as bass
    import concourse.tile as tile
    from concourse import bacc, mybir
    from concourse.masks import make_identity

    f32 = mybir.dt.float32
    f32r = mybir.dt.float32r
    bf16 = mybir.dt.bfloat16
    AF = mybir.ActivationFunctionType

    nc = bacc.Bacc("TRN2", target_bir_lowering=False, debug=False)

    x_d = nc.dram_tensor("x", [N, D], f32, kind="ExternalInput").ap()
    xq_d = nc.dram_tensor("xq", [NQ, D], f32, kind="ExternalInput").ap()
    wqkv_d = nc.dram_tensor("w_qkv", [D, 3 * D], f32, kind="ExternalInput").ap()
    # w_out feeds only f32r matmuls; declare f32r end-to-end
    wout_d = nc.dram_tensor("w_out", [D, D], f32r, kind="ExternalInput").ap()
    bout_d = nc.dram_tensor("b_out", [D], f32, kind="ExternalInput").ap()
    out_d = nc.dram_tensor("out", [NQ, D], f32, kind="ExternalOutput").ap()

    with tile.TileContext(nc) as tc:
        with (
            tc.tile_pool(name="const", bufs=1) as const,
            tc.tile_pool(name="bigsb", bufs=1) as bigsb,
            tc.tile_pool(name="stage", bufs=3) as stage,
            tc.tile_pool(name="dout", bufs=2) as dout,
            tc.tile_pool(name="lsr", bufs=2) as lsr,
        ):
            identity = const.tile([128, 128], f32, tag="identity")
            make_identity(nc, identity[:])
            identity_bf = const.tile([128, 128], bf16, tag="identity_bf")
            make_identity(nc, identity_bf[:])
            ones_neg = const.tile([128, 1], bf16, tag="ones_neg")
            nc.vector.memset(ones_neg[:], -1.0)
            b_bc = const.tile([128, D], f32, tag="b_bc")
            nc.sync.dma_start(
                out=b_bc[:],
                in_=bass.AP(
                    tensor=bout_d.tensor,
                    offset=bout_d.offset,
                    ap=[[0, 128]] + [list(p) for p in bout_d.ap],
                ),
            )
            # qkv weights: DMA f32 then cast to bf16 on-chip
            wq = []
            for j in range(4):
                w_f = stage.tile([128, 3 * D], f32, name=f"wqf{j}", tag="wqf")
                nc.sync.dma_start(out=w_f[:], in_=wqkv_d[j * 128 : (j + 1) * 128, :])
                w_t = const.tile([128, 3 * D], bf16, name=f"wq{j}", tag=f"wq{j}")
                nc.vector.tensor_copy(w_t[:], w_f[:])
                wq.append(w_t)
            wo = []
            for j in range(4):
                w_t = const.tile([128, D], f32r, name=f"wo{j}", tag=f"wo{j}")
                nc.sync.dma_start(out=w_t[:], in_=wout_d[j * 128 : (j + 1) * 128, :])
                wo.append(w_t)

            # kv_acc: [0:128, 0:512]: four [128,128] head-pair blocks of k^T v
            # (pair c's diagonal 64x64 sub-blocks are heads 2c / 2c+1; the
            # off-diagonal cross-head terms are never read).
            # [0:1, 512:1024]: -colsum(v) over all 512 v columns.
            kv_acc = const.tile([128, 1024], f32, tag="kv_acc")
            nc.vector.memset(kv_acc[:], 0.0)
            # kv_p: per-head K=128 stationary operand for OT matmuls; head h's
            # 64x64 kv block sits at rows (h%2)*64 of col block h*64, zeros
            # elsewhere so contracting against the full qT partition range only
            # picks up head h's q rows.
            kv_p = const.tile([128, 512], bf16, tag="kv_p")
            nc.vector.memset(kv_p[:], 0.0)

            kT = [bigsb.tile([128, N], bf16, name=f"kT{c}", tag=f"kT{c}") for c in range(4)]
            qT = [bigsb.tile([128, NQ], bf16, name=f"qT{c}", tag=f"qT{c}") for c in range(4)]
            xTq = [bigsb.tile([128, NQ], bf16, name=f"xTq{j}", tag=f"xTq{j}") for j in range(4)]
            OT = [bigsb.tile([128, NQ], f32r, name=f"OT{c}", tag=f"OT{c}") for c in range(4)]

            lse_acc = const.tile([128, 128], f32, tag="lse_acc")
            lse_sum = const.tile([128, 64], f32, tag="lse_sum")
            lse_ln = const.tile([128, 64], f32, tag="lse_ln")

            # ---------------- Phase A/B: transposes, k/v/kv/vsum, kT, qT -------
            with tc.tile_pool(name="ab_ps", bufs=1, space="PSUM") as abps:
                # s * x^T for own query rows (feeds the qT matmuls; folding the
                # attention scale here pre-scales q so exp/lse need scale=1)
                for tt in range(QT):
                    big = abps.tile([128, 2048], f32, name="big", tag="big", bufs=2)
                    xs = stage.tile([128, D], f32, name="x_stage", tag="x_stage")
                    nc.sync.dma_start(out=xs[:], in_=xq_d[tt * 128 : (tt + 1) * 128, :])
                    for j in range(4):
                        nc.tensor.transpose(
                            big[:, j * 128 : (j + 1) * 128],
                            xs[:, j * 128 : (j + 1) * 128],
                            identity[:],
                        )
                    for j in range(4):
                        nc.vector.tensor_scalar_mul(
                            xTq[j][:, tt * 128 : (tt + 1) * 128],
                            big[:, j * 128 : (j + 1) * 128],
                            SCALE,
                        )

                # qT: (scale*q)^T for own rows, directly transposed
                for m in range(4):
                    for nn in range(2):
                        qps = abps.tile([128, 2048], f32, name="big", tag="big", bufs=2)
                        for j in range(4):
                            nc.tensor.matmul(
                                qps[:, 0:512],
                                lhsT=wq[j][:, m * 128 : (m + 1) * 128],
                                rhs=xTq[j][:, nn * 512 : (nn + 1) * 512],
                                start=(j == 0),
                                stop=(j == 3),
                            )
                        nc.vector.tensor_copy(
                            qT[m][:, nn * 512 : (nn + 1) * 512], qps[:, 0:512]
                        )

                # full-sequence pass: k, v, kv accumulation, kT
                for t in range(NT):
                    big = abps.tile([128, 2048], f32, name="big", tag="big", bufs=2)
                    xs = stage.tile([128, D], f32, name="x_stage", tag="x_stage")
                    nc.sync.dma_start(out=xs[:], in_=x_d[t * 128 : (t + 1) * 128, :])
                    # seg0 <- x_tile^T
                    for j in range(4):
                        nc.tensor.transpose(
                            big[:, j * 128 : (j + 1) * 128],
                            xs[:, j * 128 : (j + 1) * 128],
                            identity[:],
                        )
                    xts = stage.tile([128, D], bf16, name="xT_stage", tag="xT_stage")
                    nc.vector.tensor_copy(xts[:], big[:, 0:512])
                    # seg1 <- k tile, seg2 <- v tile (natural layout)
                    for half in range(2):
                        for j in range(4):
                            nc.tensor.matmul(
                                big[:, 512 + half * 512 : 1024 + half * 512],
                                lhsT=xts[:, j * 128 : (j + 1) * 128],
                                rhs=wq[j][:, 512 + half * 512 : 1024 + half * 512],
                                start=(j == 0),
                                stop=(j == 3),
                            )
                    ks = stage.tile([128, D], bf16, name="k_stage", tag="k_stage")
                    vs = stage.tile([128, D], bf16, name="v_stage", tag="v_stage")
                    nc.vector.tensor_copy(ks[:], big[:, 512:1024])
                    nc.vector.tensor_copy(vs[:], big[:, 1024:1536])
                    # seg3 <- head-pair k^T v blocks [128, 128] x 4
                    for p in range(4):
                        nc.tensor.matmul(
                            big[:, 1536 + p * 128 : 1536 + (p + 1) * 128],
                            lhsT=ks[:, p * 128 : (p + 1) * 128],
                            rhs=vs[:, p * 128 : (p + 1) * 128],
                            start=True,
                            stop=True,
                        )
                    # seg0 reuse (bf16 view) <- k_tile^T, scatter into resident kT
                    seg0bf = big[:, 0:512].bitcast(bf16)
                    for j in range(4):
                        nc.tensor.transpose(
                            seg0bf[:, j * 128 : (j + 1) * 128],
                            ks[:, j * 128 : (j + 1) * 128],
                            identity_bf[:],
                        )
                    for j in range(4):
                        nc.vector.tensor_copy(
                            kT[j][:, t * 128 : (t + 1) * 128],
                            seg0bf[:, j * 128 : (j + 1) * 128],
                        )
                    # seg0 row 0 (after kT copies) <- -colsum(v) [1, 512]
                    nc.tensor.matmul(
                        big[0:1, 0:512], lhsT=ones_neg[:], rhs=vs[:],
                        start=True, stop=True,
                    )
                    nc.vector.tensor_add(
                        kv_acc[:, 0:512], kv_acc[:, 0:512], big[:, 1536:2048]
                    )
                    nc.vector.tensor_add(
                        kv_acc[0:1, 512:1024], kv_acc[0:1, 512:1024], big[0:1, 0:512]
                    )

                for h in range(H):
                    r0 = (h % 2) * 64
                    nc.vector.tensor_copy(
                        kv_p[r0 : r0 + 64, h * 64 : (h + 1) * 64],
                        kv_acc[r0 : r0 + 64, (h // 2) * 128 + r0 : (h // 2) * 128 + r0 + 64],
                    )

            # ---------------- Phase C: scores + exp + row-sums (lse) ----------
            with tc.tile_pool(name="c_ps", bufs=1, space="PSUM") as cps:
                for h in range(H):
                    r0 = (h % 2) * 64
                    c = h // 2
                    for t in range(QT):
                        for half in range(2):
                            dots = cps.tile(
                                [128, 2048], f32, name="dots", tag="dots", bufs=2
                            )
                            for cc in range(4):
                                nc.tensor.matmul(
                                    dots[:, cc * 512 : (cc + 1) * 512],
                                    lhsT=qT[c][r0 : r0 + 64, t * 128 : (t + 1) * 128],
                                    rhs=kT[c][
                                        r0 : r0 + 64,
                                        (half * 4 + cc) * 512 : (half * 4 + cc + 1) * 512,
                                    ],
                                    start=True,
                                    stop=True,
                                )
                            col = (h * 8 + t) * 2 + half
                            nc.scalar.activation(
                                out=dots[:],
                                in_=dots[:],
                                func=AF.Exp,
                                scale=1.0,
                                accum_out=lse_acc[:, col : col + 1],
                            )
                la = lse_acc[:].rearrange("p (c two) -> p c two", two=2)
                nc.vector.tensor_add(lse_sum[:], la[:, :, 0], la[:, :, 1])
                nc.scalar.activation(out=lse_ln[:], in_=lse_sum[:], func=AF.Ln)

            # ---------------- Phase D/E: outputs ------------------------------
            with tc.tile_pool(name="de_ps", bufs=1, space="PSUM") as deps:
                for h in range(H):
                    r0 = (h % 2) * 64
                    c = h // 2
                    # lse for this head as a single row [1, 1024] at partition 0
                    lrp = deps.tile([128, 1024], f32, name="lrp", tag="lrp", bufs=1)
                    for t in range(QT):
                        nc.tensor.transpose(
                            lrp[0:1, t * 128 : (t + 1) * 128],
                            lse_ln[:, h * 8 + t : h * 8 + t + 1],
                            identity[:],
                        )
                    lrs = lsr.tile([1, 1024], f32, name="lrs", tag="lrs")
                    nc.vector.tensor_copy(lrs[:], lrp[0:1, :])
                    # OT_h = (kv_h)^T (s q_h)^T - vsum_h (x) lse_h
                    ot = deps.tile([128, 1024], f32, name="ot", tag="ot", bufs=2)
                    for nn in range(2):
                        nc.tensor.matmul(
                            ot[r0 : r0 + 64, nn * 512 : (nn + 1) * 512],
                            lhsT=kv_p[:, h * 64 : (h + 1) * 64],
                            rhs=qT[c][:, nn * 512 : (nn + 1) * 512],
                            start=True,
                            stop=False,
                        )
                        for tt in range(4):
                            t = nn * 4 + tt
                            nc.tensor.matmul(
                                ot[r0 : r0 + 64, nn * 512 + tt * 128 : nn * 512 + (tt + 1) * 128],
                                lhsT=kv_acc[0:1, 512 + h * 64 : 512 + (h + 1) * 64],
                                rhs=lrs[0:1, t * 128 : (t + 1) * 128],
                                start=False,
                                stop=(tt == 3),
                            )
                    nc.vector.tensor_copy(OT[c][r0 : r0 + 64, :], ot[r0 : r0 + 64, :])

                # final projection + bias + residual
                for t in range(QT):
                    yps = deps.tile([128, 512], f32, name="yps", tag="yps", bufs=2)
                    for c in range(4):
                        nc.tensor.matmul(
                            yps[:],
                            lhsT=OT[c][:, t * 128 : (t + 1) * 128],
                            rhs=wo[c][:],
                            start=(c == 0),
                            stop=(c == 3),
                        )
                    xr = dout.tile([128, D], f32, name="xr", tag="xr")
                    nc.sync.dma_start(out=xr[:], in_=xq_d[t * 128 : (t + 1) * 128, :])
                    ysb = dout.tile([128, D], f32, name="ysb", tag="ysb")
                    nc.vector.tensor_add(ysb[:], yps[:], xr[:])
                    nc.vector.tensor_add(ysb[:], ysb[:], b_bc[:])
                    nc.sync.dma_start(out=out_d[t * 128 : (t + 1) * 128, :], in_=ysb[:])

    nc.compile()
    return nc


def get_graph():
    if "nc" not in _GRAPH_CACHE:
        _GRAPH_CACHE["nc"] = _build_graph()
    return _GRAPH_CACHE["nc"]


def make_in_maps(x, w_qkv, w_out, b_out):
    x = np.ascontiguousarray(x, dtype=np.float32)
    w_qkv = np.ascontiguousarray(w_qkv, dtype=np.float32)
    w_out = np.ascontiguousarray(w_out, dtype=np.float32)
    b_out = np.ascontiguousarray(b_out, dtype=np.float32)
    in_maps = []
    for i in range(8):
        b, q = divmod(i, 4)
        in_maps.append(
            {
                "x": x[b],
                "xq": np.ascontiguousarray(x[b, q * NQ : (q + 1) * NQ]),
                "w_qkv": w_qkv,
                "w_out": w_out,
                "b_out": b_out,
            }
        )
    return in_maps


def kernel(x, w_qkv, w_out, b_out):
    from concourse.bass_utils import run_bass_kernel_spmd

    nc = get_graph()
    in_maps = make_in_maps(x, w_qkv, w_out, b_out)
    res = run_bass_kernel_spmd(nc, in_maps, core_ids=list(range(8)))
    out = np.empty((B, N, D), np.float32)
    for i in range(8):
        b, q = divmod(i, 4)
        out[b, q * NQ : (q + 1) * NQ] = res.results[i]["out"]
    return out
